# revision 1
# baseline (speedup 1.0000x reference)
"""Trainium2 Bass kernel for nn_DilatedOCA (dilated overlapping cross-attention).

Math (per reference):
  xn = x / sqrt(var(x, ch) + 1e-5) * ln_w           (bias-free LN over channels)
  qkv = w_qkv @ xn (1x1 conv); q/k/v split
  q: [heads, N=4096, 16] from channels
  k,v: torch-unfold(12x12 win, stride 8, pad 2) + a reshape that scrambles
       (channel, window-pos, window-idx) into [heads, M=9216, 16] where the
       "16" dim is the LOW 4 BITS OF THE WINDOW INDEX (faithful to source).
  attn = softmax(q k^T / 4) over all M; out = attn @ v; final 1x1 conv w_out.

Index algebra (head h, d = 8a+s with a=d//8, s=d%8):
  K^T[d, m] = k_pad[16h+ci, 16lq+8a+kh, 8s+kw]   m=(ci,kh,kw,lq)
  V[m, d]   = v_pad[16h+ci, 16lq+8a+kh, 8s+kw]
  Q^T[d, n] = q[16h+d, n]
Softmax/PV are invariant to any fixed permutation of m, so we use our own
enumeration  m' = ((((khH*3+khL)*4+lq)*6+kw1)*2+kw0)*16+ci  (kh=3khH+khL,
kw=2kw1+kw0), which makes the K/V gather DMAs contiguous 192-element runs.

Sharding: 8 cores = 4 heads x 2 query-halves (n in [0,2048) / [2048,4096)).
Per core: LN (stats in transposed orientation; rstd folded into the conv
output scaling), qkv conv, K/V gather, flash-style attention in bf16 (no
max-subtraction: logits ~N(0,0.4), exp cannot overflow), per-head final-conv
partial y_h = w_out[:, head] @ out_h^T.  Host sums 4 head partials per half.
"""

import sys

for _p in ("/opt/trn_rl_repo", "/root/.axon_site/_ro/pypackages"):
    if _p not in sys.path:
        sys.path.insert(0, _p)

import numpy as np

import concourse.bass as bass
import concourse.mybir as mybir
import concourse.tile as tile
from concourse import bacc
from concourse.bass_utils import run_bass_kernel_spmd

F32 = mybir.dt.float32
BF16 = mybir.dt.float16  # fp16: same PE rate as bf16, 8x mantissa
AF = mybir.ActivationFunctionType
ALU = mybir.AluOpType

HEADS, DH = 4, 16
NPIX, NHALF = 4096, 2048
PADW = 68          # padded image height/width
PFREE = PADW * 16  # padT3 free size: col*16 + ci = 1088
M = 9216           # keys per head
NT = 72            # m' tiles of 128
EPS = 1e-5

_CACHE = {}


def _build(stage="full", reps=1):
    nc = bacc.Bacc(trn_type="TRN2")
    dbg_d = None
    if stage != "full":
        dbg_d = nc.dram_tensor("dbg", [128, M], F32, kind="ExternalOutput")

    x_d = nc.dram_tensor("x", [64, NPIX], F32, kind="ExternalInput")
    xq_d = nc.dram_tensor("xq", [64, NHALF], F32, kind="ExternalInput")
    wkvT_d = nc.dram_tensor("wkvT", [64, 32], F32, kind="ExternalInput")
    wqT_d = nc.dram_tensor("wqT", [64, 16], F32, kind="ExternalInput")
    woutT_d = nc.dram_tensor("woutT", [16, 64], F32, kind="ExternalInput")
    ones1_d = nc.dram_tensor("ones1", [1, 64], F32, kind="ExternalInput")
    id128_d = nc.dram_tensor("id128", [128, 128], F32, kind="ExternalInput")
    id17_d = nc.dram_tensor("id17", [17, 17], F32, kind="ExternalInput")
    onesM_d = nc.dram_tensor("onesM", [1, M], F32, kind="ExternalInput")
    y_d = nc.dram_tensor("y", [64, NHALF], F32, kind="ExternalOutput")
    ktmp_d = nc.dram_tensor("ktmp", [NPIX, 16], F32)
    vtmp_d = nc.dram_tensor("vtmp", [NPIX, 16], F32)

    with tile.TileContext(nc) as tc:
        with tc.tile_pool(name="sb", bufs=1) as sb:
            # persistent sbuf tensors
            xsb = sb.tile([64, NPIX], F32)
            xqsb = sb.tile([64, NHALF], F32)
            padk = sb.tile([PADW, PFREE], F32)
            padv = sb.tile([PADW, PFREE], F32)
            gk = sb.tile([16, M], F32)
            gkb = sb.tile([16, M], BF16)
            gv = sb.tile([17, M], F32)
            vt_all = sb.tile([128, 17 * NT], BF16)
            qsb = sb.tile([16, NHALF], BF16)
            stgkv = sb.tile([128, 1024], F32)
            stats = sb.tile([128, 96], F32)   # s1 cols 0:48, s2 cols 48:96
            rstdT = sb.tile([128, 48], F32)   # col t: chunk t (32 x, 16 xq)
            osb = sb.tile([128, 512], F32)
            ysb = sb.tile([64, NHALF], F32)
            wkvT = sb.tile([64, 32], F32)
            wqT = sb.tile([64, 16], F32)
            woutT = sb.tile([16, 64], F32)
            ones1 = sb.tile([1, 64], F32)
            id128 = sb.tile([128, 128], F32)
            id17 = sb.tile([17, 17], F32)

            for dst, src in (
                (xsb, x_d), (xqsb, xq_d), (wkvT, wkvT_d), (wqT, wqT_d),
                (woutT, woutT_d), (ones1, ones1_d), (id128, id128_d),
                (id17, id17_d),
            ):
                nc.sync.dma_start(out=dst[:, :], in_=src[:, :])

            # border zeros for padded images; ones row for the softmax denom
            nc.gpsimd.memset(padk[:, :], 0.0)
            nc.gpsimd.memset(padv[:, :], 0.0)
            nc.sync.dma_start(out=gv[16:17, :], in_=onesM_d[:, :])

            with tc.tile_pool(name="sm", bufs=3) as sm, \
                 tc.tile_pool(name="pre", bufs=3, space="PSUM") as pre:

                # ---- LN stats in transposed (pixel-partition) orientation --
                def chunk_src(t):
                    if t < 32:
                        return xsb[:, 128 * t:128 * (t + 1)]
                    return xqsb[:, 128 * (t - 32):128 * (t - 31)]

                for t in range(48):
                    trp = pre.tile([128, 64], F32, tag="pre")
                    nc.tensor.transpose(trp[:, :], chunk_src(t),
                                        id128[0:64, 0:64])
                    xT = sm.tile([128, 64], F32, tag="xT")
                    nc.vector.tensor_copy(xT[:, :], trp[:, :])
                    nc.vector.reduce_sum(stats[:, t:t + 1], xT[:, :],
                                         axis=mybir.AxisListType.X)
                    scr = sm.tile([128, 64], F32, tag="scr")
                    nc.vector.tensor_mul(scr[:, :], xT[:, :], xT[:, :])
                    nc.vector.reduce_sum(stats[:, 48 + t:49 + t], scr[:, :],
                                         axis=mybir.AxisListType.X)

                # rstd = 1/sqrt(s2/64 - (s1/64)^2 + eps)   [128, 48]
                mean = sm.tile([128, 48], F32, tag="mean")
                nc.vector.tensor_scalar_mul(mean[:, :], stats[:, 0:48], 1.0 / 64)
                nc.vector.tensor_mul(mean[:, :], mean[:, :], mean[:, :])
                varr = sm.tile([128, 48], F32, tag="varr")
                nc.vector.tensor_scalar_mul(varr[:, :], stats[:, 48:96], 1.0 / 64)
                nc.vector.tensor_sub(varr[:, :], varr[:, :], mean[:, :])
                nc.vector.tensor_scalar_add(varr[:, :], varr[:, :], EPS)
                nc.scalar.activation(rstdT[:, :], varr[:, :], AF.Sqrt)
                nc.vector.reciprocal(rstdT[:, :], rstdT[:, :])

                # ---- k,v 1x1 conv on RAW x; rstd folded into psum scaling --
                for t in range(32):
                    kv = pre.tile([128, 32], F32, tag="pre")
                    nc.tensor.matmul(kv[:, :], xsb[:, 128 * t:128 * (t + 1)],
                                     wkvT[:, :], start=True, stop=True)
                    nc.vector.tensor_scalar_mul(
                        stgkv[:, 32 * t:32 * (t + 1)], kv[:, :],
                        rstdT[:, t:t + 1])

                # stgkv[p, 32t + c0 + ci] = (k|v)[ci, pixel=128t+p]
                # -> (k|v)tmp[pixel, ci]  (DRAM, pixel-major)
                for tmp_d, c0 in ((ktmp_d, 0), (vtmp_d, 16)):
                    src_ap = bass.AP(tensor=stgkv.tensor, offset=c0,
                                     ap=[[1024, 128], [32, 32], [1, 16]])
                    dst_ap = bass.AP(tensor=tmp_d, offset=0,
                                     ap=[[16, 128], [2048, 32], [1, 16]])
                    nc.sync.dma_start(out=dst_ap, in_=src_ap)
                # -> pad[row, (col+2)*16 + ci] interior (rows/cols +2 offset)
                for tmp_d, pad_t in ((ktmp_d, padk), (vtmp_d, padv)):
                    src_ap = bass.AP(tensor=tmp_d, offset=0,
                                     ap=[[1024, 64], [1, 1024]])
                    dst_ap = bass.AP(tensor=pad_t.tensor,
                                     offset=2 * PFREE + 2 * 16,
                                     ap=[[PFREE, 64], [1, 1024]])
                    nc.sync.dma_start(out=dst_ap, in_=src_ap)

                if stage == "pads":
                    nc.sync.dma_start(out=bass.AP(tensor=dbg_d, offset=0,
                                                  ap=[[M, PADW], [1, PFREE]]),
                                      in_=padk[:, :])
                    nc.sync.dma_start(
                        out=bass.AP(tensor=dbg_d, offset=2048,
                                    ap=[[M, PADW], [1, PFREE]]),
                        in_=padv[:, :])

                # ---- q conv (head slice, 0.25 prefolded), pixel-part -------
                for t in range(16):
                    qp = pre.tile([128, 16], F32, tag="pre")
                    nc.tensor.matmul(qp[:, :], xqsb[:, 128 * t:128 * (t + 1)],
                                     wqT[:, :], start=True, stop=True)
                    qTc = sm.tile([128, 16], F32, tag="qTc")
                    nc.vector.tensor_scalar_mul(qTc[:, :], qp[:, :],
                                                rstdT[:, 32 + t:33 + t])
                    qp2 = pre.tile([16, 128], F32, tag="pre")
                    nc.tensor.transpose(qp2[:, :], qTc[:, :], id128[:, :])
                    nc.vector.tensor_copy(qsb[:, 128 * t:128 * (t + 1)],
                                          qp2[:, :])

                # ---- gathers: pad -> G  (48 DMAs each) ---------------------
                # G[8a+s, m'] = pad[16lq+3khH+khL+8a, (8s+2kw1+kw0)*16+ci]
                for pad_t, g_t in ((padk, gk), (padv, gv)):
                    for khH in range(4):
                        for khL in range(3):
                            for lq in range(4):
                                row0 = 16 * lq + 3 * khH + khL
                                src_ap = bass.AP(
                                    tensor=pad_t.tensor, offset=row0 * PFREE,
                                    ap=[[8 * PFREE, 2], [128, 8], [1, 192]])
                                dst_ap = bass.AP(
                                    tensor=g_t.tensor,
                                    offset=2304 * khH + 768 * khL + 192 * lq,
                                    ap=[[M, 16], [1, 192]])
                                nc.sync.dma_start(out=dst_ap, in_=src_ap)

                if stage == "gather":
                    nc.sync.dma_start(out=dbg_d[0:16, :], in_=gk[:, :])
                    nc.sync.dma_start(out=dbg_d[32:49, :], in_=gv[:, :])

                # bf16 cast of K^T for full-rate PE matmuls
                nc.vector.tensor_copy(gkb[:, :], gk[:, :])

                # ---- V^T tiles: [17, 128] -> [128, 17] via PE transpose ----
                for T in range(NT):
                    vtp = pre.tile([128, 17], F32, tag="pre")
                    nc.tensor.transpose(vtp[:, :], gv[:, 128 * T:128 * (T + 1)],
                                        id17[:, :])
                    nc.vector.tensor_copy(vt_all[:, 17 * T:17 * (T + 1)],
                                          vtp[:, :])

            if stage == "vt":
                nc.gpsimd.dma_start(out=dbg_d[:, 0:17 * NT], in_=vt_all[:, :])
            if stage in ("pads", "gather", "vt"):
                nc.vector.memset(ysb[:, :], 0.0)
                nc.sync.dma_start(out=y_d[:, :], in_=ysb[:, :])
            else:
                _run_main(nc, tc, stage, dbg_d, sb, gkb, gv, vt_all, qsb, osb,
                          ysb, woutT, ones1, y_d, reps)

    nc.compile()
    return nc


def _run_main(nc, tc, stage, dbg_d, sb, gkb, gv, vt_all, qsb, osb,
              ysb, woutT, ones1, y_d, reps=1):
    with tc.tile_pool(name="spool", bufs=2, space="PSUM") as spool, \
         tc.tile_pool(name="pvp", bufs=1, space="PSUM") as pvp, \
         tc.tile_pool(name="pp", bufs=3) as pp:
        pv = pvp.tile([128, 512], F32)
        for _rep in range(reps):
          for t3 in range(24):
            for nck in range(4):
                ncs = slice(512 * nck, 512 * (nck + 1))
                st = spool.tile([128, 1536], F32, tag="s")
                for g in range(3):
                    T = 3 * t3 + g
                    nc.tensor.matmul(
                        st[:, 512 * g:512 * (g + 1)],
                        gkb[:, 128 * T:128 * (T + 1)],
                        qsb[:, ncs], start=True, stop=True)
                pt = pp.tile([128, 1536], BF16, tag="p")
                nc.scalar.activation(pt[:, :], st[:, :], AF.Exp)
                for g in range(3):
                    T = 3 * t3 + g
                    nc.tensor.matmul(
                        pv[32 * nck:32 * nck + 17, :],
                        vt_all[:, 17 * T:17 * (T + 1)],
                        pt[:, 512 * g:512 * (g + 1)],
                        start=(t3 == 0 and g == 0),
                        stop=(t3 == 23 and g == 2),
                        tile_position=(0, 32 * nck))

        # ---- normalize + final 1x1 conv -------------------------------
        # matmul operands must sit at base partition 0: DMA each
        # accumulator strip [17, 512] down from partitions 32*nck.
        nc.vector.tensor_copy(osb[:, :], pv[:, :])
        if stage == "pv":
            nc.sync.dma_start(out=dbg_d[:, 0:512], in_=osb[:, :])
        num = sb.tile([16, 512], F32)
        den = sb.tile([1, 512], F32)
        bsb = sb.tile([64, 512], F32)
        for nck in range(4):
            src_num = bass.AP(tensor=osb.tensor, offset=32 * nck * 512,
                              ap=[[512, 16], [1, 512]])
            src_den = bass.AP(tensor=osb.tensor,
                              offset=(32 * nck + 16) * 512,
                              ap=[[512, 1], [1, 512]])
            nc.sync.dma_start(out=num[:, :], in_=src_num)
            nc.sync.dma_start(out=den[:, :], in_=src_den)
            nc.vector.reciprocal(den[:, :], den[:, :])
            yp = spool.tile([64, 512], F32, tag="s")
            nc.tensor.matmul(yp[:, :], woutT[:, :], num[:, :],
                             start=True, stop=True)
            bp = spool.tile([64, 512], F32, tag="s")
            nc.tensor.matmul(bp[:, :], ones1[:, :], den[:, :],
                             start=True, stop=True)
            # DVE has one PSUM read port: stage bp in SBUF first
            nc.vector.tensor_copy(bsb[:, :], bp[:, :])
            nc.vector.tensor_mul(ysb[:, 512 * nck:512 * (nck + 1)],
                                 yp[:, :], bsb[:, :])
        nc.sync.dma_start(out=y_d[:, :], in_=ysb[:, :])


def _get_nc():
    if "nc" not in _CACHE:
        _CACHE["nc"] = _build()
    return _CACHE["nc"]


def kernel(x, w_qkv, w_out, ln_w, _want_trace=False):
    x = np.asarray(x, np.float32)
    w_qkv = np.asarray(w_qkv, np.float32)
    w_out = np.asarray(w_out, np.float32)
    ln_w = np.asarray(ln_w, np.float32)

    x2d = np.ascontiguousarray(x.reshape(64, NPIX))
    ones1 = np.ones((1, 64), np.float32)
    id128 = np.eye(128, dtype=np.float32)
    id17 = np.eye(17, dtype=np.float32)
    onesM = np.ones((1, M), np.float32)

    in_maps = []
    for c in range(8):
        h, half = c % 4, c // 4
        wq = w_qkv[16 * h:16 * h + 16, :]
        wk = w_qkv[64 + 16 * h:64 + 16 * h + 16, :]
        wv = w_qkv[128 + 16 * h:128 + 16 * h + 16, :]
        lw = ln_w[None, :]
        in_maps.append({
            "x": x2d,
            "xq": np.ascontiguousarray(x2d[:, NHALF * half:NHALF * (half + 1)]),
            "wkvT": np.ascontiguousarray(
                (np.concatenate([wk, wv], 0) * lw).T.astype(np.float32)),
            "wqT": np.ascontiguousarray((0.25 * wq * lw).T.astype(np.float32)),
            "woutT": np.ascontiguousarray(
                w_out[:, 16 * h:16 * h + 16].T.astype(np.float32)),
            "ones1": ones1,
            "id128": id128,
            "id17": id17,
            "onesM": onesM,
        })

    nc = _get_nc()
    res = run_bass_kernel_spmd(nc, in_maps, list(range(8)), trace=_want_trace)
    if _want_trace:
        _CACHE["last_result"] = res

    y = np.empty((64, NPIX), np.float32)
    for half in range(2):
        acc = np.zeros((64, NHALF), np.float32)
        for h in range(4):
            acc += res.results[4 * half + h]["y"]
        y[:, NHALF * half:NHALF * (half + 1)] = acc
    return y.reshape(1, 64, 64, 64)

